# revision 4
# baseline (speedup 1.0000x reference)
"""Davis-Yin splitting LP solver kernel for Trainium2 (8 NeuronCores, data parallel).

Math per batch item (B=256 total, 32 per core):
  A = [As | I]  (128 x 640),  P = As_inv = pinv(A)  (640 x 128)
  iterate 50x (step i, alpha a):
    p2 = relu(s)
    t  = (2-a)*p2 - s - a*c
    r  = As @ t[:512] + t[512:] - b          (down-projection, 128)
    u  = As_inv @ r                          (up-projection, 640)
    s  = (s - p2 + t) - u  ==  (1-a)*p2 - a*c - u       (algebraic identity)
  out = relu(s)

Device layout (per core):
  - State vectors in "column layout": SBUF [128 partitions, nb*5 cols],
    col (b*5+k) holds elements [128k : 128(k+1)) of item b's 640-vector.
  - Down-proj weights: AsT chunks, lhsT_k[dk, m] = As[m, 128k+dk] (4 per item).
  - Up-proj weights: Pinv chunks, lhsT_j[k, d'] = As_inv[128j+d', k] (5 per item).
  - All matvecs are PE matmuls with the matrix as the bf16 stationary operand
    and an N=1 moving vector (the weight-load path is the PE bottleneck; bf16
    costs ~1e-3 accuracy vs the 2e-2 budget).
  - Elementwise work is split across DVE (critical path: q, t, r, snew),
    ACT (p2s and e1 relu-scales), and GpSimd (w), batched per half-group
    (nh=2 -> 16 items) so it overlaps PE work.
  - The two halves run skewed by `lag` iterations so the PE can chew on
    half A's early iterations while half B's weights are still streaming
    from HBM.
"""

import numpy as np

import concourse.bass as bass
import concourse.mybir as mybir
from concourse.tile import TileContext
from concourse.bass_utils import run_bass_kernel_spmd

F32 = mybir.dt.float32
BF16 = mybir.dt.bfloat16
AF = mybir.ActivationFunctionType
ALU = mybir.AluOpType

B, M, N = 256, 128, 512
D = M + N  # 640
NCORES = 8
NB = B // NCORES  # 32 items per core
NUM_ITER = 50
ALPHA, TAU, DECAY = 0.05, 1.0, 10.0


def _alphas(num_iter):
    i = np.arange(num_iter, dtype=np.float32)
    base = np.float32(1.0) - i / np.float32(NUM_ITER)
    return (np.float32(ALPHA) * base ** (np.float32(1.0) / np.float32(DECAY))).astype(
        np.float32
    )


def _legalize_waits_json(raw: bytes) -> bytes:
    """Walrus (this revision) accepts at most 1 sync-wait per instruction
    (2 for EventSemaphore), but Tile emits up to 2 on compute instructions.
    Hoist excess waits onto standalone EventSemaphore instructions inserted
    just before the over-subscribed instruction (same engine, so the waits
    still happen-before it in queue order)."""
    import json as _json

    bir = _json.loads(raw)
    ctr = [0]

    def process_block(instrs):
        out = []
        for inst in instrs:
            si = inst.get("sync_info")
            if si:
                waits = si.get("on_wait") or []
                cap = 2 if inst.get("opcode") == "EventSemaphore" else 1
                if len(waits) > cap:
                    extra, keep = waits[:-cap], waits[-cap:]
                    for i in range(0, len(extra), 2):
                        ctr[0] += 1
                        out.append(
                            {
                                "debug": inst.get("debug", 0),
                                "engine": inst["engine"],
                                "ins": [],
                                "name": f"waitfix_{ctr[0]}",
                                "opcode": "EventSemaphore",
                                "outs": [],
                                "sync_info": {
                                    "on_update": [],
                                    "on_wait": extra[i : i + 2],
                                },
                            }
                        )
                    si["on_wait"] = keep
            out.append(inst)
        return out

    def walk(o):
        if isinstance(o, dict):
            for k, v in o.items():
                if k == "instructions" and isinstance(v, list):
                    o[k] = process_block(v)
                else:
                    walk(v)
        elif isinstance(o, list):
            for v in o:
                walk(v)

    walk(bir)
    return _json.dumps(bir).encode()


def _patch_serialization(nc):
    orig = nc.to_json_bytes

    def patched():
        return _legalize_waits_json(orig())

    nc.to_json_bytes = patched
    return nc


def build_program(nb=NB, num_iter=NUM_ITER, nh=2, lag=2, wdt=BF16):
    """Build the per-core Bass program (identical across cores)."""
    nc = bass.Bass(use_seq_codegen=True, num_swdge_queues=4)
    AsT_d = nc.dram_tensor("AsT", [nb, 4, 128, 128], wdt, kind="ExternalInput")
    Pinv_d = nc.dram_tensor("Pinv", [nb, 5, 128, 128], wdt, kind="ExternalInput")
    c_d = nc.dram_tensor("ccol", [128, nb * 5], F32, kind="ExternalInput")
    b_d = nc.dram_tensor("bcol", [128, nb], F32, kind="ExternalInput")
    out_d = nc.dram_tensor("out", [128, nb * 5], F32, kind="ExternalOutput")

    alphas = _alphas(num_iter)
    hs = nb // nh  # items per half-group

    with TileContext(nc) as tc:
        with (
            tc.tile_pool(name="wpool", bufs=1) as wpool,
            tc.tile_pool(name="spool", bufs=3) as spool,
            tc.tile_pool(name="tpool", bufs=3) as tpool,
            tc.tile_pool(name="ppool", bufs=2, space="PSUM") as ppool,
        ):
            ccol = wpool.tile([128, nb * 5], F32, tag="ccol")
            bcol = wpool.tile([128, nb], F32, tag="bcol")
            nc.sync.dma_start(out=ccol[:], in_=c_d[:])
            nc.sync.dma_start(out=bcol[:], in_=b_d[:])
            # Per-item weight tiles, issued half-A items first so the skewed
            # schedule can start on half A while half B still streams in.
            AsT_t, Pinv_t = [None] * nb, [None] * nb
            for b in range(nb):
                at = wpool.tile([128, 4 * 128], wdt, tag=f"AsT{b}")
                pv = wpool.tile([128, 5 * 128], wdt, tag=f"Pinv{b}")
                eng_a = nc.sync if b % 2 == 0 else nc.gpsimd
                eng_b = nc.gpsimd if b % 2 == 0 else nc.sync
                eng_a.dma_start(
                    out=at[:].rearrange("p (k j) -> p k j", k=4),
                    in_=AsT_d[b].rearrange("k i j -> i k j"),
                )
                eng_b.dma_start(
                    out=pv[:].rearrange("p (k j) -> p k j", k=5),
                    in_=Pinv_d[b].rearrange("k i j -> i k j"),
                )
                AsT_t[b] = at
                Pinv_t[b] = pv

            states = [None] * nh
            preps = [None] * nh

            def emit_prep(h, sh, a):
                """Elementwise for the NEXT iteration of half h, given new
                state sh.  Returns (t_mm, tsb, w)."""
                sl = slice(h * hs * 5, (h + 1) * hs * 5)
                slb = slice(h * hs, (h + 1) * hs)
                p2s = tpool.tile([128, hs * 5], F32, tag=f"p2s{h}")
                q = tpool.tile([128, hs * 5], F32, tag=f"q{h}")
                e1 = tpool.tile([128, hs * 5], F32, tag=f"e1{h}")
                t_mm = tpool.tile([128, hs * 5], wdt, tag=f"t{h}")
                w = tpool.tile([128, hs * 5], F32, tag=f"w{h}")
                tsb = tpool.tile([128, hs], F32, tag=f"tsb{h}")

                # p2s = (2-a)*relu(s) = relu((2-a)*s)   [ACT]
                nc.scalar.activation(p2s[:], sh[:], AF.Relu, scale=float(2.0 - a))
                # q = a*c + s                            [DVE]
                nc.vector.scalar_tensor_tensor(
                    q[:], ccol[:, sl], float(a), sh[:], op0=ALU.mult, op1=ALU.add
                )
                # e1 = (1-a)*relu(s) = relu((1-a)*s)     [ACT]
                nc.scalar.activation(e1[:], sh[:], AF.Relu, scale=float(1.0 - a))
                # t = p2s - q  (bf16, feeds the PE)      [DVE]
                nc.vector.tensor_sub(t_mm[:], p2s[:], q[:])
                # tsb = t_slack - b                      [DVE]
                nc.vector.tensor_sub(tsb[:], t_mm[:, 4::5], bcol[:, slb])
                # w = e1 - a*c  (= s - p2 + t)           [DVE, off critical path]
                nc.vector.scalar_tensor_tensor(
                    w[:], ccol[:, sl], float(-a), e1[:], op0=ALU.mult, op1=ALU.add
                )
                return t_mm, tsb, w

            def emit_prep0(h):
                """Iteration-0 elementwise: s=0, so t = w = -a0*c."""
                a0 = float(alphas[0])
                sl = slice(h * hs * 5, (h + 1) * hs * 5)
                slb = slice(h * hs, (h + 1) * hs)
                t_mm = tpool.tile([128, hs * 5], wdt, tag=f"t{h}")
                w = tpool.tile([128, hs * 5], F32, tag=f"w{h}")
                tsb = tpool.tile([128, hs], F32, tag=f"tsb{h}")
                nc.vector.tensor_scalar(t_mm[:], ccol[:, sl], -a0, 0.0, op0=ALU.mult)
                nc.vector.tensor_sub(tsb[:], t_mm[:, 4::5], bcol[:, slb])
                nc.vector.tensor_scalar(w[:], ccol[:, sl], -a0, 0.0, op0=ALU.mult)
                return t_mm, tsb, w

            def emit_down(h):
                t_mm = preps[h][0]
                psum_y = ppool.tile([128, hs], F32, tag=f"py{h}")
                for bi in range(hs):
                    bg = h * hs + bi
                    for k in range(4):
                        nc.tensor.matmul(
                            psum_y[:, bi : bi + 1],
                            lhsT=AsT_t[bg][:, k * 128 : (k + 1) * 128],
                            rhs=t_mm[:, bi * 5 + k : bi * 5 + k + 1],
                            start=(k == 0),
                            stop=(k == 3),
                        )
                return psum_y

            def emit_r(h, psum_y):
                tsb = preps[h][1]
                r_mm = tpool.tile([128, hs], wdt, tag=f"rbf{h}")
                nc.vector.tensor_add(r_mm[:], psum_y[:], tsb[:])
                return r_mm

            def emit_up(h, r_mm):
                psum_u = ppool.tile([128, 5 * hs], F32, tag=f"pu{h}")
                for bi in range(hs):
                    bg = h * hs + bi
                    for j in range(5):
                        nc.tensor.matmul(
                            psum_u[:, bi * 5 + j : bi * 5 + j + 1],
                            lhsT=Pinv_t[bg][:, j * 128 : (j + 1) * 128],
                            rhs=r_mm[:, bi : bi + 1],
                            start=True,
                            stop=True,
                        )
                return psum_u

            final = wpool.tile([128, nb * 5], F32, tag="final")

            def emit_snew(h, psum_u, it):
                w = preps[h][2]
                s_new = spool.tile([128, hs * 5], F32, tag=f"state{h}")
                nc.vector.tensor_sub(s_new[:], w[:], psum_u[:])
                states[h] = s_new
                if it + 1 < num_iter:
                    preps[h] = emit_prep(h, s_new, float(alphas[it + 1]))
                else:
                    # Final relu + output DMA for this half, immediately.
                    sl = slice(h * hs * 5, (h + 1) * hs * 5)
                    nc.scalar.activation(final[:, sl], s_new[:], AF.Relu)
                    nc.sync.dma_start(out=out_d[:, sl], in_=final[:, sl])

            # Skewed schedule: half 0 runs iteration i while half 1 runs
            # iteration i-lag.  Each step interleaves down/up of the two
            # halves (down0, down1, up0, up1) so the PE->DVE->PE roundtrip
            # of one half hides behind the other half's matmuls.
            assert nh == 2
            preps[0] = emit_prep0(0)
            preps[1] = emit_prep0(1)

            for i in range(num_iter + lag):
                ia, ib = i, i - lag
                run_a, run_b = ia < num_iter, 0 <= ib < num_iter
                py_a = emit_down(0) if run_a else None
                if run_a:
                    r_a = emit_r(0, py_a)
                py_b = emit_down(1) if run_b else None
                if run_a:
                    pu_a = emit_up(0, r_a)
                if run_b:
                    r_b = emit_r(1, py_b)
                if run_a:
                    emit_snew(0, pu_a, ia)
                if run_b:
                    pu_b = emit_up(1, r_b)
                    emit_snew(1, pu_b, ib)

    return _patch_serialization(nc)


def _prep_core_inputs(c_input, As, bs, As_inv, nb, np_wdt):
    """Host-side marshaling of one core's shard into the device layouts."""
    AsT = np.ascontiguousarray(
        As.reshape(nb, 128, 4, 128).transpose(0, 2, 3, 1)
    ).astype(np_wdt)
    Pinv = np.ascontiguousarray(
        As_inv.reshape(nb, 5, 128, 128).transpose(0, 1, 3, 2)
    ).astype(np_wdt)
    ccol = np.ascontiguousarray(
        c_input.reshape(nb, 5, 128).transpose(2, 0, 1).reshape(128, nb * 5),
        dtype=np.float32,
    )
    bcol = np.ascontiguousarray(bs.T, dtype=np.float32)
    return {"AsT": AsT, "Pinv": Pinv, "ccol": ccol, "bcol": bcol}


def kernel(c_input, As, bs, As_inv, _trace=False, _nc_cache={}):
    import ml_dtypes

    c_input = np.asarray(c_input, dtype=np.float32)
    As = np.asarray(As, dtype=np.float32)
    bs = np.asarray(bs, dtype=np.float32)
    As_inv = np.asarray(As_inv, dtype=np.float32)

    np_wdt = ml_dtypes.bfloat16
    if "nc" not in _nc_cache:
        _nc_cache["nc"] = build_program()
    nc = _nc_cache["nc"]

    in_maps = []
    for core in range(NCORES):
        sl = slice(core * NB, (core + 1) * NB)
        in_maps.append(
            _prep_core_inputs(
                c_input[sl], As[sl], bs[sl], As_inv[sl], NB, np_wdt=np_wdt
            )
        )

    res = run_bass_kernel_spmd(nc, in_maps, core_ids=list(range(NCORES)), trace=_trace)

    out = np.empty((B, D), dtype=np.float32)
    for core in range(NCORES):
        oc = res.results[core]["out"]  # [128, NB*5]
        out[core * NB : (core + 1) * NB] = (
            oc.reshape(128, NB, 5).transpose(1, 2, 0).reshape(NB, D)
        )
    if _trace:
        kernel.last_exec_time_ns = res.exec_time_ns
    return out


# revision 6
# speedup vs baseline: 1.2038x; 1.2038x over previous
"""Davis-Yin splitting LP solver kernel for Trainium2 (8 NeuronCores, data parallel).

Math per batch item (B=256 total, 32 per core):
  A = [As | I]  (128 x 640),  P = As_inv = pinv(A)  (640 x 128)
  iterate 50x (step i, alpha a):
    p2 = relu(s)
    t  = (2-a)*p2 - s - a*c
    r  = As @ t[:512] + t[512:] - b          (down-projection, 128)
    u  = As_inv @ r                          (up-projection, 640)
    s  = (s - p2 + t) - u  ==  (1-a)*p2 - a*c - u       (algebraic identity)
  out = relu(s)

Device layout (per core):
  - State vectors in "column layout": SBUF [128 partitions, nb*5 cols],
    col (b*5+k) holds elements [128k : 128(k+1)) of item b's 640-vector.
  - Down-proj weights: AsT chunks, lhsT_k[dk, m] = As[m, 128k+dk] (4 per item).
  - Up-proj weights: Pinv chunks, lhsT_j[k, d'] = As_inv[128j+d', k] (5 per item).
  - All matvecs are PE matmuls with the matrix as the bf16 stationary operand
    and an N=1 moving vector (the weight-load path is the PE bottleneck; bf16
    costs ~1e-3 accuracy vs the 2e-2 budget).
  - Elementwise work is split across DVE (critical path: q, t, r, snew),
    ACT (p2s and e1 relu-scales), and GpSimd (w), batched per half-group
    (nh=2 -> 16 items) so it overlaps PE work.
  - The two halves run skewed by `lag` iterations so the PE can chew on
    half A's early iterations while half B's weights are still streaming
    from HBM.
"""

import numpy as np

import concourse.bass as bass
import concourse.mybir as mybir
from concourse.tile import TileContext
from concourse.bass_utils import run_bass_kernel_spmd

F32 = mybir.dt.float32
BF16 = mybir.dt.bfloat16
AF = mybir.ActivationFunctionType
ALU = mybir.AluOpType

B, M, N = 256, 128, 512
D = M + N  # 640
NCORES = 8
NB = B // NCORES  # 32 items per core
NUM_ITER = 50
ALPHA, TAU, DECAY = 0.05, 1.0, 10.0


def _alphas(num_iter):
    i = np.arange(num_iter, dtype=np.float32)
    base = np.float32(1.0) - i / np.float32(NUM_ITER)
    return (np.float32(ALPHA) * base ** (np.float32(1.0) / np.float32(DECAY))).astype(
        np.float32
    )


def _strip_unused_sem_incs(bir):
    """Engine semaphore increments retire serially at ~34ns each — far slower
    than the ~27ns matmul pair rate — so per-instruction sem-incs both lag
    (delaying dependent engines by the backlog) and throttle the PE.  Since
    each engine completes instructions in order, only increments whose
    cumulative count is actually awaited are needed.  Keep exactly those,
    drop the rest, and renumber every wait to the kept-inc rank.

    Only applied to semaphores that are (a) incremented exclusively by
    instructions of a single compute engine with sem-inc/+1, (b) never
    updated by DMA (queue completion order != engine program order), and
    (c) only ever waited on with sem-ge-imm."""
    # Pass 1: gather per-sem info in program order.
    blocks = []

    def collect_blocks(o):
        if isinstance(o, dict):
            for k, v in o.items():
                if k == "instructions" and isinstance(v, list):
                    blocks.append(v)
                else:
                    collect_blocks(v)
        elif isinstance(o, list):
            for v in o:
                collect_blocks(v)

    collect_blocks(bir)

    upd_engines = {}   # sem id -> set of engines that update it
    upd_ok = {}        # sem id -> all updates are sem-inc +1 non-DMA
    wait_ok = {}       # sem id -> all waits are sem-ge-imm
    wait_values = {}   # sem id -> set of awaited values
    for instrs in blocks:
        for inst in instrs:
            si = inst.get("sync_info")
            if not si:
                continue
            is_dma = "DMA" in (inst.get("opcode") or "")
            for u in si.get("on_update") or []:
                if u.get("sync_type") != "semaphore":
                    continue
                sid = u["id"]
                upd_engines.setdefault(sid, set()).add(inst.get("engine"))
                ok = (
                    u.get("update_mode") == "sem-inc"
                    and u.get("update_value") == 1
                    and not is_dma
                )
                upd_ok[sid] = upd_ok.get(sid, True) and ok
            for w in si.get("on_wait") or []:
                if w.get("sync_type") != "semaphore":
                    continue
                sid = w["id"]
                wait_ok[sid] = wait_ok.get(sid, True) and (
                    w.get("wait_mode") == "sem-ge-imm"
                )
                wait_values.setdefault(sid, set()).add(w["wait_value"])

    target = {
        sid
        for sid, engs in upd_engines.items()
        if len(engs) == 1
        and upd_ok.get(sid, False)
        and wait_ok.get(sid, True)
    }

    # Count total incs per sem first, so the final inc can always be kept
    # (insurance for any exit logic polling the terminal count).
    totals = {sid: 0 for sid in target}
    for instrs in blocks:
        for inst in instrs:
            si = inst.get("sync_info")
            if not si:
                continue
            for u in si.get("on_update") or []:
                sid = u.get("id")
                if u.get("sync_type") == "semaphore" and sid in target:
                    totals[sid] += 1
    keep_values = {
        sid: (wait_values.get(sid, set()) | {totals[sid]}) for sid in target
    }

    # Pass 2: drop unneeded incs; build old-count -> rank map per sem.
    counters = {sid: 0 for sid in target}
    kept_sorted = {sid: sorted(keep_values[sid]) for sid in target}
    for instrs in blocks:
        for inst in instrs:
            si = inst.get("sync_info")
            if not si or not si.get("on_update"):
                continue
            new_upd = []
            for u in si["on_update"]:
                sid = u.get("id")
                if u.get("sync_type") == "semaphore" and sid in target:
                    counters[sid] += 1
                    if counters[sid] in keep_values[sid]:
                        new_upd.append(u)
                else:
                    new_upd.append(u)
            si["on_update"] = new_upd

    # sanity: every awaited value must be <= total inc count
    for sid in target:
        vals = kept_sorted[sid]
        if vals and vals[-1] > counters[sid]:
            raise RuntimeError(
                f"sem {sid}: awaited {vals[-1]} > total incs {counters[sid]}"
            )

    # Pass 3: renumber waits to rank within kept values.
    import bisect

    for instrs in blocks:
        for inst in instrs:
            si = inst.get("sync_info")
            if not si:
                continue
            for w in si.get("on_wait") or []:
                sid = w.get("id")
                if w.get("sync_type") == "semaphore" and sid in target:
                    w["wait_value"] = (
                        bisect.bisect_right(kept_sorted[sid], w["wait_value"])
                    )
    return bir


def _legalize_waits_json(raw: bytes) -> bytes:
    """Walrus (this revision) accepts at most 1 sync-wait per instruction
    (2 for EventSemaphore), but Tile emits up to 2 on compute instructions.
    Hoist excess waits onto standalone EventSemaphore instructions inserted
    just before the over-subscribed instruction (same engine, so the waits
    still happen-before it in queue order)."""
    import json as _json

    bir = _json.loads(raw)
    bir = _strip_unused_sem_incs(bir)
    ctr = [0]

    def process_block(instrs):
        out = []
        for inst in instrs:
            si = inst.get("sync_info")
            if si:
                waits = si.get("on_wait") or []
                cap = 2 if inst.get("opcode") == "EventSemaphore" else 1
                if len(waits) > cap:
                    extra, keep = waits[:-cap], waits[-cap:]
                    for i in range(0, len(extra), 2):
                        ctr[0] += 1
                        out.append(
                            {
                                "debug": inst.get("debug", 0),
                                "engine": inst["engine"],
                                "ins": [],
                                "name": f"waitfix_{ctr[0]}",
                                "opcode": "EventSemaphore",
                                "outs": [],
                                "sync_info": {
                                    "on_update": [],
                                    "on_wait": extra[i : i + 2],
                                },
                            }
                        )
                    si["on_wait"] = keep
            out.append(inst)
        return out

    def walk(o):
        if isinstance(o, dict):
            for k, v in o.items():
                if k == "instructions" and isinstance(v, list):
                    o[k] = process_block(v)
                else:
                    walk(v)
        elif isinstance(o, list):
            for v in o:
                walk(v)

    walk(bir)
    return _json.dumps(bir).encode()


def _patch_serialization(nc):
    orig = nc.to_json_bytes

    def patched():
        return _legalize_waits_json(orig())

    nc.to_json_bytes = patched
    return nc


def build_program(nb=NB, num_iter=NUM_ITER, nh=2, lag=2, wdt=BF16):
    """Build the per-core Bass program (identical across cores)."""
    nc = bass.Bass(use_seq_codegen=True, num_swdge_queues=4)
    AsT_d = nc.dram_tensor("AsT", [nb, 4, 128, 128], wdt, kind="ExternalInput")
    Pinv_d = nc.dram_tensor("Pinv", [nb, 5, 128, 128], wdt, kind="ExternalInput")
    c_d = nc.dram_tensor("ccol", [128, nb * 5], F32, kind="ExternalInput")
    b_d = nc.dram_tensor("bcol", [128, nb], F32, kind="ExternalInput")
    out_d = nc.dram_tensor("out", [128, nb * 5], F32, kind="ExternalOutput")

    alphas = _alphas(num_iter)
    hs = nb // nh  # items per half-group

    with TileContext(nc) as tc:
        with (
            tc.tile_pool(name="wpool", bufs=1) as wpool,
            tc.tile_pool(name="spool", bufs=3) as spool,
            tc.tile_pool(name="tpool", bufs=3) as tpool,
            tc.tile_pool(name="ppool", bufs=2, space="PSUM") as ppool,
        ):
            ccol = wpool.tile([128, nb * 5], F32, tag="ccol")
            bcol = wpool.tile([128, nb], F32, tag="bcol")
            nc.sync.dma_start(out=ccol[:], in_=c_d[:])
            nc.sync.dma_start(out=bcol[:], in_=b_d[:])
            # Per-item weight tiles, issued half-A items first so the skewed
            # schedule can start on half A while half B still streams in.
            AsT_t, Pinv_t = [None] * nb, [None] * nb
            for b in range(nb):
                at = wpool.tile([128, 4 * 128], wdt, tag=f"AsT{b}")
                pv = wpool.tile([128, 5 * 128], wdt, tag=f"Pinv{b}")
                eng_a = nc.sync if b % 2 == 0 else nc.gpsimd
                eng_b = nc.gpsimd if b % 2 == 0 else nc.sync
                eng_a.dma_start(
                    out=at[:].rearrange("p (k j) -> p k j", k=4),
                    in_=AsT_d[b].rearrange("k i j -> i k j"),
                )
                eng_b.dma_start(
                    out=pv[:].rearrange("p (k j) -> p k j", k=5),
                    in_=Pinv_d[b].rearrange("k i j -> i k j"),
                )
                AsT_t[b] = at
                Pinv_t[b] = pv

            states = [None] * nh
            preps = [None] * nh

            def emit_prep(h, sh, a):
                """Elementwise for the NEXT iteration of half h, given new
                state sh.  Returns (t_mm, tsb, w)."""
                sl = slice(h * hs * 5, (h + 1) * hs * 5)
                slb = slice(h * hs, (h + 1) * hs)
                p2s = tpool.tile([128, hs * 5], F32, tag=f"p2s{h}")
                q = tpool.tile([128, hs * 5], F32, tag=f"q{h}")
                e1 = tpool.tile([128, hs * 5], F32, tag=f"e1{h}")
                t_mm = tpool.tile([128, hs * 5], wdt, tag=f"t{h}")
                w = tpool.tile([128, hs * 5], F32, tag=f"w{h}")
                tsb = tpool.tile([128, hs], F32, tag=f"tsb{h}")

                # p2s = (2-a)*relu(s) = relu((2-a)*s)   [ACT]
                nc.scalar.activation(p2s[:], sh[:], AF.Relu, scale=float(2.0 - a))
                # q = a*c + s                            [DVE]
                nc.vector.scalar_tensor_tensor(
                    q[:], ccol[:, sl], float(a), sh[:], op0=ALU.mult, op1=ALU.add
                )
                # e1 = (1-a)*relu(s) = relu((1-a)*s)     [ACT]
                nc.scalar.activation(e1[:], sh[:], AF.Relu, scale=float(1.0 - a))
                # t = p2s - q  (bf16, feeds the PE)      [DVE]
                nc.vector.tensor_sub(t_mm[:], p2s[:], q[:])
                # tsb = t_slack - b                      [DVE]
                nc.vector.tensor_sub(tsb[:], t_mm[:, 4::5], bcol[:, slb])
                # w = e1 - a*c  (= s - p2 + t)           [DVE, off critical path]
                nc.vector.scalar_tensor_tensor(
                    w[:], ccol[:, sl], float(-a), e1[:], op0=ALU.mult, op1=ALU.add
                )
                return t_mm, tsb, w

            def emit_prep0(h):
                """Iteration-0 elementwise: s=0, so t = w = -a0*c."""
                a0 = float(alphas[0])
                sl = slice(h * hs * 5, (h + 1) * hs * 5)
                slb = slice(h * hs, (h + 1) * hs)
                t_mm = tpool.tile([128, hs * 5], wdt, tag=f"t{h}")
                w = tpool.tile([128, hs * 5], F32, tag=f"w{h}")
                tsb = tpool.tile([128, hs], F32, tag=f"tsb{h}")
                nc.vector.tensor_scalar(t_mm[:], ccol[:, sl], -a0, 0.0, op0=ALU.mult)
                nc.vector.tensor_sub(tsb[:], t_mm[:, 4::5], bcol[:, slb])
                nc.vector.tensor_scalar(w[:], ccol[:, sl], -a0, 0.0, op0=ALU.mult)
                return t_mm, tsb, w

            def emit_down(h):
                t_mm = preps[h][0]
                psum_y = ppool.tile([128, hs], F32, tag=f"py{h}")
                for bi in range(hs):
                    bg = h * hs + bi
                    for k in range(4):
                        nc.tensor.matmul(
                            psum_y[:, bi : bi + 1],
                            lhsT=AsT_t[bg][:, k * 128 : (k + 1) * 128],
                            rhs=t_mm[:, bi * 5 + k : bi * 5 + k + 1],
                            start=(k == 0),
                            stop=(k == 3),
                        )
                return psum_y

            def emit_r(h, psum_y):
                tsb = preps[h][1]
                r_mm = tpool.tile([128, hs], wdt, tag=f"rbf{h}")
                nc.vector.tensor_add(r_mm[:], psum_y[:], tsb[:])
                return r_mm

            def emit_up(h, r_mm):
                psum_u = ppool.tile([128, 5 * hs], F32, tag=f"pu{h}")
                for bi in range(hs):
                    bg = h * hs + bi
                    for j in range(5):
                        nc.tensor.matmul(
                            psum_u[:, bi * 5 + j : bi * 5 + j + 1],
                            lhsT=Pinv_t[bg][:, j * 128 : (j + 1) * 128],
                            rhs=r_mm[:, bi : bi + 1],
                            start=True,
                            stop=True,
                        )
                return psum_u

            final = wpool.tile([128, nb * 5], F32, tag="final")

            def emit_snew(h, psum_u, it):
                w = preps[h][2]
                s_new = spool.tile([128, hs * 5], F32, tag=f"state{h}")
                nc.vector.tensor_sub(s_new[:], w[:], psum_u[:])
                states[h] = s_new
                if it + 1 < num_iter:
                    preps[h] = emit_prep(h, s_new, float(alphas[it + 1]))
                else:
                    # Final relu + output DMA for this half, immediately.
                    sl = slice(h * hs * 5, (h + 1) * hs * 5)
                    nc.scalar.activation(final[:, sl], s_new[:], AF.Relu)
                    nc.sync.dma_start(out=out_d[:, sl], in_=final[:, sl])

            # Skewed schedule: half 0 runs iteration i while half 1 runs
            # iteration i-lag.  Each step interleaves down/up of the two
            # halves (down0, down1, up0, up1) so the PE->DVE->PE roundtrip
            # of one half hides behind the other half's matmuls.
            assert nh == 2
            preps[0] = emit_prep0(0)
            preps[1] = emit_prep0(1)

            for i in range(num_iter + lag):
                ia, ib = i, i - lag
                run_a, run_b = ia < num_iter, 0 <= ib < num_iter
                py_a = emit_down(0) if run_a else None
                if run_a:
                    r_a = emit_r(0, py_a)
                py_b = emit_down(1) if run_b else None
                if run_a:
                    pu_a = emit_up(0, r_a)
                if run_b:
                    r_b = emit_r(1, py_b)
                if run_a:
                    emit_snew(0, pu_a, ia)
                if run_b:
                    pu_b = emit_up(1, r_b)
                    emit_snew(1, pu_b, ib)

    return _patch_serialization(nc)


def _prep_core_inputs(c_input, As, bs, As_inv, nb, np_wdt):
    """Host-side marshaling of one core's shard into the device layouts."""
    AsT = np.ascontiguousarray(
        As.reshape(nb, 128, 4, 128).transpose(0, 2, 3, 1)
    ).astype(np_wdt)
    Pinv = np.ascontiguousarray(
        As_inv.reshape(nb, 5, 128, 128).transpose(0, 1, 3, 2)
    ).astype(np_wdt)
    ccol = np.ascontiguousarray(
        c_input.reshape(nb, 5, 128).transpose(2, 0, 1).reshape(128, nb * 5),
        dtype=np.float32,
    )
    bcol = np.ascontiguousarray(bs.T, dtype=np.float32)
    return {"AsT": AsT, "Pinv": Pinv, "ccol": ccol, "bcol": bcol}


def kernel(c_input, As, bs, As_inv, _trace=False, _nc_cache={}):
    import ml_dtypes

    c_input = np.asarray(c_input, dtype=np.float32)
    As = np.asarray(As, dtype=np.float32)
    bs = np.asarray(bs, dtype=np.float32)
    As_inv = np.asarray(As_inv, dtype=np.float32)

    np_wdt = ml_dtypes.bfloat16
    if "nc" not in _nc_cache:
        _nc_cache["nc"] = build_program()
    nc = _nc_cache["nc"]

    in_maps = []
    for core in range(NCORES):
        sl = slice(core * NB, (core + 1) * NB)
        in_maps.append(
            _prep_core_inputs(
                c_input[sl], As[sl], bs[sl], As_inv[sl], NB, np_wdt=np_wdt
            )
        )

    res = run_bass_kernel_spmd(nc, in_maps, core_ids=list(range(NCORES)), trace=_trace)

    out = np.empty((B, D), dtype=np.float32)
    for core in range(NCORES):
        oc = res.results[core]["out"]  # [128, NB*5]
        out[core * NB : (core + 1) * NB] = (
            oc.reshape(128, NB, 5).transpose(1, 2, 0).reshape(NB, D)
        )
    if _trace:
        kernel.last_exec_time_ns = res.exec_time_ns
    return out


# revision 10
# speedup vs baseline: 1.2252x; 1.0178x over previous
"""Davis-Yin splitting LP solver kernel for Trainium2 (8 NeuronCores, data parallel).

Math per batch item (B=256 total, 32 per core):
  A = [As | I]  (128 x 640),  P = As_inv = pinv(A)  (640 x 128)
  iterate 50x (step i, alpha a):
    p2 = relu(s)
    t  = (2-a)*p2 - s - a*c
    r  = As @ t[:512] + t[512:] - b          (down-projection, 128)
    u  = As_inv @ r                          (up-projection, 640)
    s  = (s - p2 + t) - u  ==  (1-a)*p2 - a*c - u       (algebraic identity)
  out = relu(s)

Device layout (per core):
  - State vectors in "column layout": SBUF [128 partitions, nb*5 cols],
    col (b*5+k) holds elements [128k : 128(k+1)) of item b's 640-vector.
  - Down-proj weights: AsT chunks, lhsT_k[dk, m] = As[m, 128k+dk] (4 per item).
  - Up-proj weights: Pinv chunks, lhsT_j[k, d'] = As_inv[128j+d', k] (5 per item).
  - All matvecs are PE matmuls with the matrix as the bf16 stationary operand
    and an N=1 moving vector (the weight-load path is the PE bottleneck; bf16
    costs ~1e-3 accuracy vs the 2e-2 budget).
  - Elementwise work is split across DVE (critical path: q, t, r, snew),
    ACT (p2s and e1 relu-scales), and GpSimd (w), batched per half-group
    (nh=2 -> 16 items) so it overlaps PE work.
  - The two halves run skewed by `lag` iterations so the PE can chew on
    half A's early iterations while half B's weights are still streaming
    from HBM.
"""

import numpy as np

import concourse.bass as bass
import concourse.mybir as mybir
from concourse.tile import TileContext
from concourse.bass_utils import run_bass_kernel_spmd

F32 = mybir.dt.float32
BF16 = mybir.dt.bfloat16
AF = mybir.ActivationFunctionType
ALU = mybir.AluOpType

B, M, N = 256, 128, 512
D = M + N  # 640
NCORES = 8
NB = B // NCORES  # 32 items per core
NUM_ITER = 50
ALPHA, TAU, DECAY = 0.05, 1.0, 10.0


def _alphas(num_iter):
    i = np.arange(num_iter, dtype=np.float32)
    base = np.float32(1.0) - i / np.float32(NUM_ITER)
    return (np.float32(ALPHA) * base ** (np.float32(1.0) / np.float32(DECAY))).astype(
        np.float32
    )


def _strip_unused_sem_incs(bir):
    """Engine semaphore increments retire serially at ~34ns each — far slower
    than the ~27ns matmul pair rate — so per-instruction sem-incs both lag
    (delaying dependent engines by the backlog) and throttle the PE.  Since
    each engine completes instructions in order, only increments whose
    cumulative count is actually awaited are needed.  Keep exactly those,
    drop the rest, and renumber every wait to the kept-inc rank.

    Only applied to semaphores that are (a) incremented exclusively by
    instructions of a single compute engine with sem-inc/+1, (b) never
    updated by DMA (queue completion order != engine program order), and
    (c) only ever waited on with sem-ge-imm."""
    # Pass 1: gather per-sem info in program order.
    blocks = []

    def collect_blocks(o):
        if isinstance(o, dict):
            for k, v in o.items():
                if k == "instructions" and isinstance(v, list):
                    blocks.append(v)
                else:
                    collect_blocks(v)
        elif isinstance(o, list):
            for v in o:
                collect_blocks(v)

    collect_blocks(bir)

    upd_engines = {}   # sem id -> set of engines that update it
    upd_ok = {}        # sem id -> all updates are sem-inc +1 non-DMA
    wait_ok = {}       # sem id -> all waits are sem-ge-imm
    wait_values = {}   # sem id -> set of awaited values
    for instrs in blocks:
        for inst in instrs:
            si = inst.get("sync_info")
            if not si:
                continue
            is_dma = "DMA" in (inst.get("opcode") or "")
            for u in si.get("on_update") or []:
                if u.get("sync_type") != "semaphore":
                    continue
                sid = u["id"]
                upd_engines.setdefault(sid, set()).add(inst.get("engine"))
                ok = (
                    u.get("update_mode") == "sem-inc"
                    and u.get("update_value") == 1
                    and not is_dma
                )
                upd_ok[sid] = upd_ok.get(sid, True) and ok
            for w in si.get("on_wait") or []:
                if w.get("sync_type") != "semaphore":
                    continue
                sid = w["id"]
                wait_ok[sid] = wait_ok.get(sid, True) and (
                    w.get("wait_mode") == "sem-ge-imm"
                )
                wait_values.setdefault(sid, set()).add(w["wait_value"])

    target = {
        sid
        for sid, engs in upd_engines.items()
        if len(engs) == 1
        and upd_ok.get(sid, False)
        and wait_ok.get(sid, True)
    }

    # Count total incs per sem first, so the final inc can always be kept
    # (insurance for any exit logic polling the terminal count).
    totals = {sid: 0 for sid in target}
    for instrs in blocks:
        for inst in instrs:
            si = inst.get("sync_info")
            if not si:
                continue
            for u in si.get("on_update") or []:
                sid = u.get("id")
                if u.get("sync_type") == "semaphore" and sid in target:
                    totals[sid] += 1
    keep_values = {
        sid: (wait_values.get(sid, set()) | {totals[sid]}) for sid in target
    }

    # Pass 2: drop unneeded incs; build old-count -> rank map per sem.
    counters = {sid: 0 for sid in target}
    kept_sorted = {sid: sorted(keep_values[sid]) for sid in target}
    for instrs in blocks:
        for inst in instrs:
            si = inst.get("sync_info")
            if not si or not si.get("on_update"):
                continue
            new_upd = []
            for u in si["on_update"]:
                sid = u.get("id")
                if u.get("sync_type") == "semaphore" and sid in target:
                    counters[sid] += 1
                    if counters[sid] in keep_values[sid]:
                        new_upd.append(u)
                else:
                    new_upd.append(u)
            si["on_update"] = new_upd

    # sanity: every awaited value must be <= total inc count
    for sid in target:
        vals = kept_sorted[sid]
        if vals and vals[-1] > counters[sid]:
            raise RuntimeError(
                f"sem {sid}: awaited {vals[-1]} > total incs {counters[sid]}"
            )

    # Pass 3: renumber waits to rank within kept values.
    import bisect

    for instrs in blocks:
        for inst in instrs:
            si = inst.get("sync_info")
            if not si:
                continue
            for w in si.get("on_wait") or []:
                sid = w.get("id")
                if w.get("sync_type") == "semaphore" and sid in target:
                    w["wait_value"] = (
                        bisect.bisect_right(kept_sorted[sid], w["wait_value"])
                    )
    return bir


def _legalize_waits_json(raw: bytes) -> bytes:
    """Walrus (this revision) accepts at most 1 sync-wait per instruction
    (2 for EventSemaphore), but Tile emits up to 2 on compute instructions.
    Hoist excess waits onto standalone EventSemaphore instructions inserted
    just before the over-subscribed instruction (same engine, so the waits
    still happen-before it in queue order)."""
    import json as _json

    bir = _json.loads(raw)
    bir = _strip_unused_sem_incs(bir)
    ctr = [0]

    def process_block(instrs):
        out = []
        for inst in instrs:
            si = inst.get("sync_info")
            if si:
                waits = si.get("on_wait") or []
                cap = 2 if inst.get("opcode") == "EventSemaphore" else 1
                if len(waits) > cap:
                    extra, keep = waits[:-cap], waits[-cap:]
                    for i in range(0, len(extra), 2):
                        ctr[0] += 1
                        out.append(
                            {
                                "debug": inst.get("debug", 0),
                                "engine": inst["engine"],
                                "ins": [],
                                "name": f"waitfix_{ctr[0]}",
                                "opcode": "EventSemaphore",
                                "outs": [],
                                "sync_info": {
                                    "on_update": [],
                                    "on_wait": extra[i : i + 2],
                                },
                            }
                        )
                    si["on_wait"] = keep
            out.append(inst)
        return out

    def walk(o):
        if isinstance(o, dict):
            for k, v in o.items():
                if k == "instructions" and isinstance(v, list):
                    o[k] = process_block(v)
                else:
                    walk(v)
        elif isinstance(o, list):
            for v in o:
                walk(v)

    walk(bir)
    return _json.dumps(bir).encode()


def _patch_serialization(nc):
    orig = nc.to_json_bytes

    def patched():
        return _legalize_waits_json(orig())

    nc.to_json_bytes = patched
    return nc


def build_program(nb=NB, num_iter=NUM_ITER, nh=4, lag=4, wdt=BF16):
    """Build the per-core Bass program (identical across cores)."""
    nc = bass.Bass(use_seq_codegen=True, num_swdge_queues=4)
    AsT_d = nc.dram_tensor("AsT", [nb, 4, 128, 128], wdt, kind="ExternalInput")
    Pinv_d = nc.dram_tensor("Pinv", [nb, 5, 128, 128], wdt, kind="ExternalInput")
    c_d = nc.dram_tensor("ccol", [128, nb * 5], F32, kind="ExternalInput")
    b_d = nc.dram_tensor("bcol", [128, nb], F32, kind="ExternalInput")
    out_d = nc.dram_tensor("out", [128, nb * 5], F32, kind="ExternalOutput")

    alphas = _alphas(num_iter)
    hs = nb // nh  # items per half-group

    with TileContext(nc) as tc:
        with (
            tc.tile_pool(name="wpool", bufs=1) as wpool,
            tc.tile_pool(name="spool", bufs=3) as spool,
            tc.tile_pool(name="tpool", bufs=3) as tpool,
            tc.tile_pool(name="ppool", bufs=1, space="PSUM") as ppool,
        ):
            ccol = wpool.tile([128, nb * 5], F32, tag="ccol")
            bcol = wpool.tile([128, nb], F32, tag="bcol")
            nc.sync.dma_start(out=ccol[:], in_=c_d[:])
            nc.sync.dma_start(out=bcol[:], in_=b_d[:])
            # Per-item weight tiles, issued half-A items first so the skewed
            # schedule can start on half A while half B still streams in.
            AsT_t, Pinv_t = [None] * nb, [None] * nb
            for b in range(nb):
                at = wpool.tile([128, 4 * 128], wdt, tag=f"AsT{b}")
                pv = wpool.tile([128, 5 * 128], wdt, tag=f"Pinv{b}")
                eng_a = nc.sync if b % 2 == 0 else nc.gpsimd
                eng_b = nc.gpsimd if b % 2 == 0 else nc.sync
                eng_a.dma_start(
                    out=at[:].rearrange("p (k j) -> p k j", k=4),
                    in_=AsT_d[b].rearrange("k i j -> i k j"),
                )
                eng_b.dma_start(
                    out=pv[:].rearrange("p (k j) -> p k j", k=5),
                    in_=Pinv_d[b].rearrange("k i j -> i k j"),
                )
                AsT_t[b] = at
                Pinv_t[b] = pv

            states = [None] * nh
            preps = [None] * nh

            def emit_prep(h, sh, a):
                """Elementwise for the NEXT iteration of half h, given new
                state sh.  Returns (t_mm, tsb, w)."""
                sl = slice(h * hs * 5, (h + 1) * hs * 5)
                slb = slice(h * hs, (h + 1) * hs)
                p2s = tpool.tile([128, hs * 5], F32, tag=f"p2s{h}")
                q = tpool.tile([128, hs * 5], F32, tag=f"q{h}")
                e1 = tpool.tile([128, hs * 5], F32, tag=f"e1{h}")
                t_mm = tpool.tile([128, hs * 5], wdt, tag=f"t{h}")
                w = tpool.tile([128, hs * 5], F32, tag=f"w{h}")
                tsb = tpool.tile([128, hs], F32, tag=f"tsb{h}")

                # p2s = (2-a)*relu(s)  (fused max+mult)  [DVE, critical]
                nc.vector.tensor_scalar(
                    p2s[:], sh[:], 0.0, float(2.0 - a), op0=ALU.max, op1=ALU.mult
                )
                # q = a*c + s                            [DVE]
                nc.vector.scalar_tensor_tensor(
                    q[:], ccol[:, sl], float(a), sh[:], op0=ALU.mult, op1=ALU.add
                )
                # e1 = (1-a)*relu(s) = relu((1-a)*s)     [ACT]
                nc.scalar.activation(e1[:], sh[:], AF.Relu, scale=float(1.0 - a))
                # t = p2s - q  (bf16, feeds the PE)      [DVE]
                nc.vector.tensor_sub(t_mm[:], p2s[:], q[:])
                # tsb = t_slack - b                      [DVE]
                nc.vector.tensor_sub(tsb[:], t_mm[:, 4::5], bcol[:, slb])
                # w = e1 - a*c  (= s - p2 + t)           [DVE, off critical path]
                nc.vector.scalar_tensor_tensor(
                    w[:], ccol[:, sl], float(-a), e1[:], op0=ALU.mult, op1=ALU.add
                )
                return t_mm, tsb, w

            def emit_prep0(h):
                """Iteration-0 elementwise: s=0, so t = w = -a0*c."""
                a0 = float(alphas[0])
                sl = slice(h * hs * 5, (h + 1) * hs * 5)
                slb = slice(h * hs, (h + 1) * hs)
                t_mm = tpool.tile([128, hs * 5], wdt, tag=f"t{h}")
                w = tpool.tile([128, hs * 5], F32, tag=f"w{h}")
                tsb = tpool.tile([128, hs], F32, tag=f"tsb{h}")
                nc.vector.tensor_scalar(t_mm[:], ccol[:, sl], -a0, 0.0, op0=ALU.mult)
                nc.vector.tensor_sub(tsb[:], t_mm[:, 4::5], bcol[:, slb])
                nc.vector.tensor_scalar(w[:], ccol[:, sl], -a0, 0.0, op0=ALU.mult)
                return t_mm, tsb, w

            def emit_down(h):
                t_mm = preps[h][0]
                psum_y = ppool.tile([128, hs], F32, tag=f"py{h}")
                for bi in range(hs):
                    bg = h * hs + bi
                    for k in range(4):
                        nc.tensor.matmul(
                            psum_y[:, bi : bi + 1],
                            lhsT=AsT_t[bg][:, k * 128 : (k + 1) * 128],
                            rhs=t_mm[:, bi * 5 + k : bi * 5 + k + 1],
                            start=(k == 0),
                            stop=(k == 3),
                        )
                return psum_y

            def emit_r(h, psum_y):
                tsb = preps[h][1]
                r_mm = tpool.tile([128, hs], wdt, tag=f"rbf{h}")
                nc.vector.tensor_add(r_mm[:], psum_y[:], tsb[:])
                return r_mm

            def emit_up(h, r_mm):
                psum_u = ppool.tile([128, 5 * hs], F32, tag=f"pu{h}")
                for bi in range(hs):
                    bg = h * hs + bi
                    for j in range(5):
                        nc.tensor.matmul(
                            psum_u[:, bi * 5 + j : bi * 5 + j + 1],
                            lhsT=Pinv_t[bg][:, j * 128 : (j + 1) * 128],
                            rhs=r_mm[:, bi : bi + 1],
                            start=True,
                            stop=True,
                        )
                return psum_u

            final = wpool.tile([128, nb * 5], F32, tag="final")

            def emit_snew(h, psum_u, it):
                w = preps[h][2]
                s_new = spool.tile([128, hs * 5], F32, tag=f"state{h}")
                nc.vector.tensor_sub(s_new[:], w[:], psum_u[:])
                states[h] = s_new
                if it + 1 < num_iter:
                    preps[h] = emit_prep(h, s_new, float(alphas[it + 1]))
                else:
                    # Final relu + output DMA for this half, immediately.
                    sl = slice(h * hs * 5, (h + 1) * hs * 5)
                    nc.scalar.activation(final[:, sl], s_new[:], AF.Relu)
                    nc.sync.dma_start(out=out_d[:, sl], in_=final[:, sl])

            # Skewed schedule: pair (h0,h1) runs iteration i while pair
            # (h2,h3) runs iteration i-lag (covers the initial weight DMA).
            # Within a pair, down/up interleave (downA downB upA upB) so the
            # PE->DVE->PE roundtrip of one half hides behind the other; the
            # snew->prep chain of a pair hides behind the *other* pair.
            assert nh == 4
            for h in range(nh):
                preps[h] = emit_prep0(h)

            def emit_pair_step(A, Bq, it):
                py_a = emit_down(A)
                r_a = emit_r(A, py_a)
                py_b = emit_down(Bq)
                pu_a = emit_up(A, r_a)
                r_b = emit_r(Bq, py_b)
                emit_snew(A, pu_a, it)
                pu_b = emit_up(Bq, r_b)
                emit_snew(Bq, pu_b, it)

            for i in range(num_iter + lag):
                if i < num_iter:
                    emit_pair_step(0, 1, i)
                ib = i - lag
                if 0 <= ib < num_iter:
                    emit_pair_step(2, 3, ib)

    return _patch_serialization(nc)


def _prep_core_inputs(c_input, As, bs, As_inv, nb, np_wdt):
    """Host-side marshaling of one core's shard into the device layouts."""
    AsT = np.ascontiguousarray(
        As.reshape(nb, 128, 4, 128).transpose(0, 2, 3, 1)
    ).astype(np_wdt)
    Pinv = np.ascontiguousarray(
        As_inv.reshape(nb, 5, 128, 128).transpose(0, 1, 3, 2)
    ).astype(np_wdt)
    ccol = np.ascontiguousarray(
        c_input.reshape(nb, 5, 128).transpose(2, 0, 1).reshape(128, nb * 5),
        dtype=np.float32,
    )
    bcol = np.ascontiguousarray(bs.T, dtype=np.float32)
    return {"AsT": AsT, "Pinv": Pinv, "ccol": ccol, "bcol": bcol}


def kernel(c_input, As, bs, As_inv, _trace=False, _nc_cache={}):
    import ml_dtypes

    c_input = np.asarray(c_input, dtype=np.float32)
    As = np.asarray(As, dtype=np.float32)
    bs = np.asarray(bs, dtype=np.float32)
    As_inv = np.asarray(As_inv, dtype=np.float32)

    np_wdt = ml_dtypes.bfloat16
    if "nc" not in _nc_cache:
        _nc_cache["nc"] = build_program()
    nc = _nc_cache["nc"]

    in_maps = []
    for core in range(NCORES):
        sl = slice(core * NB, (core + 1) * NB)
        in_maps.append(
            _prep_core_inputs(
                c_input[sl], As[sl], bs[sl], As_inv[sl], NB, np_wdt=np_wdt
            )
        )

    res = run_bass_kernel_spmd(nc, in_maps, core_ids=list(range(NCORES)), trace=_trace)

    out = np.empty((B, D), dtype=np.float32)
    for core in range(NCORES):
        oc = res.results[core]["out"]  # [128, NB*5]
        out[core * NB : (core + 1) * NB] = (
            oc.reshape(128, NB, 5).transpose(1, 2, 0).reshape(NB, D)
        )
    if _trace:
        kernel.last_exec_time_ns = res.exec_time_ns
    return out


# revision 11
# speedup vs baseline: 1.2485x; 1.0190x over previous
"""Davis-Yin splitting LP solver kernel for Trainium2 (8 NeuronCores, data parallel).

Math per batch item (B=256 total, 32 per core):
  A = [As | I]  (128 x 640),  P = As_inv = pinv(A)  (640 x 128)
  iterate 50x (step i, alpha a):
    p2 = relu(s)
    t  = (2-a)*p2 - s - a*c
    r  = As @ t[:512] + t[512:] - b          (down-projection, 128)
    u  = As_inv @ r                          (up-projection, 640)
    s  = (s - p2 + t) - u  ==  (1-a)*p2 - a*c - u       (algebraic identity)
  out = relu(s)

Device layout (per core):
  - State vectors in "column layout": SBUF [128 partitions, nb*5 cols],
    col (b*5+k) holds elements [128k : 128(k+1)) of item b's 640-vector.
  - Down-proj weights: AsT chunks, lhsT_k[dk, m] = As[m, 128k+dk] (4 per item).
  - Up-proj weights: Pinv chunks, lhsT_j[k, d'] = As_inv[128j+d', k] (5 per item).
  - All matvecs are PE matmuls with the matrix as the bf16 stationary operand
    and an N=1 moving vector (the weight-load path is the PE bottleneck; bf16
    costs ~1e-3 accuracy vs the 2e-2 budget).
  - Elementwise work is split across DVE (critical path: q, t, r, snew),
    ACT (p2s and e1 relu-scales), and GpSimd (w), batched per half-group
    (nh=2 -> 16 items) so it overlaps PE work.
  - The two halves run skewed by `lag` iterations so the PE can chew on
    half A's early iterations while half B's weights are still streaming
    from HBM.
"""

import numpy as np

import concourse.bass as bass
import concourse.mybir as mybir
from concourse.tile import TileContext
from concourse.bass_utils import run_bass_kernel_spmd

F32 = mybir.dt.float32
BF16 = mybir.dt.bfloat16
AF = mybir.ActivationFunctionType
ALU = mybir.AluOpType

B, M, N = 256, 128, 512
D = M + N  # 640
NCORES = 8
NB = B // NCORES  # 32 items per core
NUM_ITER = 50
ALPHA, TAU, DECAY = 0.05, 1.0, 10.0


def _alphas(num_iter):
    i = np.arange(num_iter, dtype=np.float32)
    base = np.float32(1.0) - i / np.float32(NUM_ITER)
    return (np.float32(ALPHA) * base ** (np.float32(1.0) / np.float32(DECAY))).astype(
        np.float32
    )


def _strip_unused_sem_incs(bir):
    """Engine semaphore increments retire serially at ~34ns each — far slower
    than the ~27ns matmul pair rate — so per-instruction sem-incs both lag
    (delaying dependent engines by the backlog) and throttle the PE.  Since
    each engine completes instructions in order, only increments whose
    cumulative count is actually awaited are needed.  Keep exactly those,
    drop the rest, and renumber every wait to the kept-inc rank.

    Only applied to semaphores that are (a) incremented exclusively by
    instructions of a single compute engine with sem-inc/+1, (b) never
    updated by DMA (queue completion order != engine program order), and
    (c) only ever waited on with sem-ge-imm."""
    # Pass 1: gather per-sem info in program order.
    blocks = []

    def collect_blocks(o):
        if isinstance(o, dict):
            for k, v in o.items():
                if k == "instructions" and isinstance(v, list):
                    blocks.append(v)
                else:
                    collect_blocks(v)
        elif isinstance(o, list):
            for v in o:
                collect_blocks(v)

    collect_blocks(bir)

    upd_engines = {}   # sem id -> set of engines that update it
    upd_ok = {}        # sem id -> all updates are sem-inc +1 non-DMA
    wait_ok = {}       # sem id -> all waits are sem-ge-imm
    wait_values = {}   # sem id -> set of awaited values
    for instrs in blocks:
        for inst in instrs:
            si = inst.get("sync_info")
            if not si:
                continue
            is_dma = "DMA" in (inst.get("opcode") or "")
            for u in si.get("on_update") or []:
                if u.get("sync_type") != "semaphore":
                    continue
                sid = u["id"]
                upd_engines.setdefault(sid, set()).add(inst.get("engine"))
                ok = (
                    u.get("update_mode") == "sem-inc"
                    and u.get("update_value") == 1
                    and not is_dma
                )
                upd_ok[sid] = upd_ok.get(sid, True) and ok
            for w in si.get("on_wait") or []:
                if w.get("sync_type") != "semaphore":
                    continue
                sid = w["id"]
                wait_ok[sid] = wait_ok.get(sid, True) and (
                    w.get("wait_mode") == "sem-ge-imm"
                )
                wait_values.setdefault(sid, set()).add(w["wait_value"])

    target = {
        sid
        for sid, engs in upd_engines.items()
        if len(engs) == 1
        and upd_ok.get(sid, False)
        and wait_ok.get(sid, True)
    }

    # Count total incs per sem first, so the final inc can always be kept
    # (insurance for any exit logic polling the terminal count).
    totals = {sid: 0 for sid in target}
    for instrs in blocks:
        for inst in instrs:
            si = inst.get("sync_info")
            if not si:
                continue
            for u in si.get("on_update") or []:
                sid = u.get("id")
                if u.get("sync_type") == "semaphore" and sid in target:
                    totals[sid] += 1
    keep_values = {
        sid: (wait_values.get(sid, set()) | {totals[sid]}) for sid in target
    }

    # Pass 2: drop unneeded incs; build old-count -> rank map per sem.
    counters = {sid: 0 for sid in target}
    kept_sorted = {sid: sorted(keep_values[sid]) for sid in target}
    for instrs in blocks:
        for inst in instrs:
            si = inst.get("sync_info")
            if not si or not si.get("on_update"):
                continue
            new_upd = []
            for u in si["on_update"]:
                sid = u.get("id")
                if u.get("sync_type") == "semaphore" and sid in target:
                    counters[sid] += 1
                    if counters[sid] in keep_values[sid]:
                        new_upd.append(u)
                else:
                    new_upd.append(u)
            si["on_update"] = new_upd

    # sanity: every awaited value must be <= total inc count
    for sid in target:
        vals = kept_sorted[sid]
        if vals and vals[-1] > counters[sid]:
            raise RuntimeError(
                f"sem {sid}: awaited {vals[-1]} > total incs {counters[sid]}"
            )

    # Pass 3: renumber waits to rank within kept values.
    import bisect

    for instrs in blocks:
        for inst in instrs:
            si = inst.get("sync_info")
            if not si:
                continue
            for w in si.get("on_wait") or []:
                sid = w.get("id")
                if w.get("sync_type") == "semaphore" and sid in target:
                    w["wait_value"] = (
                        bisect.bisect_right(kept_sorted[sid], w["wait_value"])
                    )
    return bir


def _legalize_waits_json(raw: bytes) -> bytes:
    """Walrus (this revision) accepts at most 1 sync-wait per instruction
    (2 for EventSemaphore), but Tile emits up to 2 on compute instructions.
    Hoist excess waits onto standalone EventSemaphore instructions inserted
    just before the over-subscribed instruction (same engine, so the waits
    still happen-before it in queue order)."""
    import json as _json

    bir = _json.loads(raw)
    bir = _strip_unused_sem_incs(bir)
    ctr = [0]

    def process_block(instrs):
        out = []
        for inst in instrs:
            si = inst.get("sync_info")
            if si:
                waits = si.get("on_wait") or []
                cap = 2 if inst.get("opcode") == "EventSemaphore" else 1
                if len(waits) > cap:
                    extra, keep = waits[:-cap], waits[-cap:]
                    for i in range(0, len(extra), 2):
                        ctr[0] += 1
                        out.append(
                            {
                                "debug": inst.get("debug", 0),
                                "engine": inst["engine"],
                                "ins": [],
                                "name": f"waitfix_{ctr[0]}",
                                "opcode": "EventSemaphore",
                                "outs": [],
                                "sync_info": {
                                    "on_update": [],
                                    "on_wait": extra[i : i + 2],
                                },
                            }
                        )
                    si["on_wait"] = keep
            out.append(inst)
        return out

    def walk(o):
        if isinstance(o, dict):
            for k, v in o.items():
                if k == "instructions" and isinstance(v, list):
                    o[k] = process_block(v)
                else:
                    walk(v)
        elif isinstance(o, list):
            for v in o:
                walk(v)

    walk(bir)
    return _json.dumps(bir).encode()


def _patch_serialization(nc):
    orig = nc.to_json_bytes

    def patched():
        return _legalize_waits_json(orig())

    nc.to_json_bytes = patched
    return nc


def build_program(nb=NB, num_iter=NUM_ITER, nh=4, lag=4, wdt=BF16):
    """Build the per-core Bass program (identical across cores)."""
    nc = bass.Bass(use_seq_codegen=True, num_swdge_queues=4)
    AsT_d = nc.dram_tensor("AsT", [nb, 4, 128, 128], wdt, kind="ExternalInput")
    Pinv_d = nc.dram_tensor("Pinv", [nb, 5, 128, 128], wdt, kind="ExternalInput")
    c_d = nc.dram_tensor("ccol", [128, nb * 5], F32, kind="ExternalInput")
    b_d = nc.dram_tensor("bcol", [128, nb], F32, kind="ExternalInput")
    out_d = nc.dram_tensor("out", [128, nb * 5], F32, kind="ExternalOutput")

    alphas = _alphas(num_iter)
    hs = nb // nh  # items per half-group

    with TileContext(nc) as tc:
        with (
            tc.tile_pool(name="wpool", bufs=1) as wpool,
            tc.tile_pool(name="spool", bufs=3) as spool,
            tc.tile_pool(name="tpool", bufs=3) as tpool,
            tc.tile_pool(name="ppool", bufs=1, space="PSUM") as ppool,
        ):
            ccol = wpool.tile([128, nb * 5], F32, tag="ccol")
            bcol = wpool.tile([128, nb], F32, tag="bcol")
            nc.sync.dma_start(out=ccol[:], in_=c_d[:])
            nc.sync.dma_start(out=bcol[:], in_=b_d[:])
            # Per-item weight tiles, issued half-A items first so the skewed
            # schedule can start on half A while half B still streams in.
            AsT_t, Pinv_t = [None] * nb, [None] * nb
            for b in range(nb):
                at = wpool.tile([128, 4 * 128], wdt, tag=f"AsT{b}")
                pv = wpool.tile([128, 5 * 128], wdt, tag=f"Pinv{b}")
                eng_a = nc.sync if b % 2 == 0 else nc.gpsimd
                eng_b = nc.gpsimd if b % 2 == 0 else nc.sync
                eng_a.dma_start(
                    out=at[:].rearrange("p (k j) -> p k j", k=4),
                    in_=AsT_d[b].rearrange("k i j -> i k j"),
                )
                eng_b.dma_start(
                    out=pv[:].rearrange("p (k j) -> p k j", k=5),
                    in_=Pinv_d[b].rearrange("k i j -> i k j"),
                )
                AsT_t[b] = at
                Pinv_t[b] = pv

            states = [None] * nh
            preps = [None] * nh

            def emit_prep(h, sh, a):
                """Elementwise for the NEXT iteration of half h, given new
                state sh.  Returns (t_mm, tsb, w)."""
                sl = slice(h * hs * 5, (h + 1) * hs * 5)
                slb = slice(h * hs, (h + 1) * hs)
                p2s = tpool.tile([128, hs * 5], F32, tag=f"p2s{h}")
                q = tpool.tile([128, hs * 5], F32, tag=f"q{h}")
                e1 = tpool.tile([128, hs * 5], F32, tag=f"e1{h}")
                t_mm = tpool.tile([128, hs * 5], wdt, tag=f"t{h}")
                w = tpool.tile([128, hs * 5], F32, tag=f"w{h}")
                tsb = tpool.tile([128, hs], F32, tag=f"tsb{h}")

                # p2s = (2-a)*relu(s)  (fused max+mult)  [DVE, critical]
                nc.vector.tensor_scalar(
                    p2s[:], sh[:], 0.0, float(2.0 - a), op0=ALU.max, op1=ALU.mult
                )
                # q = a*c + s                            [DVE]
                nc.vector.scalar_tensor_tensor(
                    q[:], ccol[:, sl], float(a), sh[:], op0=ALU.mult, op1=ALU.add
                )
                # e1 = (1-a)*relu(s) = relu((1-a)*s)     [ACT]
                nc.scalar.activation(e1[:], sh[:], AF.Relu, scale=float(1.0 - a))
                # t = p2s - q  (bf16, feeds the PE)      [DVE]
                nc.vector.tensor_sub(t_mm[:], p2s[:], q[:])
                # tsb = t_slack - b                      [DVE]
                nc.vector.tensor_sub(tsb[:], t_mm[:, 4::5], bcol[:, slb])
                # w = e1 - a*c  (= s - p2 + t)           [DVE, off critical path]
                nc.vector.scalar_tensor_tensor(
                    w[:], ccol[:, sl], float(-a), e1[:], op0=ALU.mult, op1=ALU.add
                )
                return t_mm, tsb, w

            def emit_prep0(h):
                """Iteration-0 elementwise: s=0, so t = w = -a0*c."""
                a0 = float(alphas[0])
                sl = slice(h * hs * 5, (h + 1) * hs * 5)
                slb = slice(h * hs, (h + 1) * hs)
                t_mm = tpool.tile([128, hs * 5], wdt, tag=f"t{h}")
                w = tpool.tile([128, hs * 5], F32, tag=f"w{h}")
                tsb = tpool.tile([128, hs], F32, tag=f"tsb{h}")
                nc.vector.tensor_scalar(t_mm[:], ccol[:, sl], -a0, 0.0, op0=ALU.mult)
                nc.vector.tensor_sub(tsb[:], t_mm[:, 4::5], bcol[:, slb])
                nc.vector.tensor_scalar(w[:], ccol[:, sl], -a0, 0.0, op0=ALU.mult)
                return t_mm, tsb, w

            def emit_down(h):
                t_mm = preps[h][0]
                psum_y = ppool.tile([128, hs], F32, tag=f"py{h}")
                for bi in range(hs):
                    bg = h * hs + bi
                    for k in range(4):
                        nc.tensor.matmul(
                            psum_y[:, bi : bi + 1],
                            lhsT=AsT_t[bg][:, k * 128 : (k + 1) * 128],
                            rhs=t_mm[:, bi * 5 + k : bi * 5 + k + 1],
                            start=(k == 0),
                            stop=(k == 3),
                        )
                return psum_y

            def emit_r(h, psum_y):
                tsb = preps[h][1]
                r_mm = tpool.tile([128, hs], wdt, tag=f"rbf{h}")
                nc.vector.tensor_add(r_mm[:], psum_y[:], tsb[:])
                return r_mm

            def emit_up(h, r_mm):
                psum_u = ppool.tile([128, 5 * hs], F32, tag=f"pu{h}")
                for bi in range(hs):
                    bg = h * hs + bi
                    for j in range(5):
                        nc.tensor.matmul(
                            psum_u[:, bi * 5 + j : bi * 5 + j + 1],
                            lhsT=Pinv_t[bg][:, j * 128 : (j + 1) * 128],
                            rhs=r_mm[:, bi : bi + 1],
                            start=True,
                            stop=True,
                        )
                return psum_u

            final = wpool.tile([128, nb * 5], F32, tag="final")

            def emit_snew(h, psum_u, it):
                w = preps[h][2]
                s_new = spool.tile([128, hs * 5], F32, tag=f"state{h}")
                nc.vector.tensor_sub(s_new[:], w[:], psum_u[:])
                states[h] = s_new
                if it + 1 < num_iter:
                    preps[h] = emit_prep(h, s_new, float(alphas[it + 1]))
                else:
                    # Final relu + output DMA for this half, immediately.
                    sl = slice(h * hs * 5, (h + 1) * hs * 5)
                    nc.scalar.activation(final[:, sl], s_new[:], AF.Relu)
                    nc.sync.dma_start(out=out_d[:, sl], in_=final[:, sl])

            # Skewed schedule: pair (h0,h1) runs iteration i while pair
            # (h2,h3) runs iteration i-lag (covers the initial weight DMA).
            # Within a pair, down/up interleave (downA downB upA upB) so the
            # PE->DVE->PE roundtrip of one half hides behind the other; the
            # snew->prep chain of a pair hides behind the *other* pair.
            assert nh == 4
            for h in range(nh):
                preps[h] = emit_prep0(h)

            # The Tile scheduler follows CoreSim's simulated timing, which
            # models neither the LDW bandwidth nor semaphore serialization,
            # and so reorders the phases badly.  Stamp each phase group with
            # a strictly increasing wait-ts to force the intended PE order.
            stamp_ctr = [0]

            def stamped(fn, *args):
                stamp_ctr[0] += 1
                with tc.tile_wait_until(stamp_ctr[0]):
                    return fn(*args)

            def emit_pair_step(A, Bq, it):
                py_a = stamped(emit_down, A)
                r_a = stamped(emit_r, A, py_a)
                py_b = stamped(emit_down, Bq)
                pu_a = stamped(emit_up, A, r_a)
                r_b = stamped(emit_r, Bq, py_b)
                stamped(emit_snew, A, pu_a, it)
                pu_b = stamped(emit_up, Bq, r_b)
                stamped(emit_snew, Bq, pu_b, it)

            for i in range(num_iter + lag):
                if i < num_iter:
                    emit_pair_step(0, 1, i)
                ib = i - lag
                if 0 <= ib < num_iter:
                    emit_pair_step(2, 3, ib)

    return _patch_serialization(nc)


def _prep_core_inputs(c_input, As, bs, As_inv, nb, np_wdt):
    """Host-side marshaling of one core's shard into the device layouts."""
    AsT = np.ascontiguousarray(
        As.reshape(nb, 128, 4, 128).transpose(0, 2, 3, 1)
    ).astype(np_wdt)
    Pinv = np.ascontiguousarray(
        As_inv.reshape(nb, 5, 128, 128).transpose(0, 1, 3, 2)
    ).astype(np_wdt)
    ccol = np.ascontiguousarray(
        c_input.reshape(nb, 5, 128).transpose(2, 0, 1).reshape(128, nb * 5),
        dtype=np.float32,
    )
    bcol = np.ascontiguousarray(bs.T, dtype=np.float32)
    return {"AsT": AsT, "Pinv": Pinv, "ccol": ccol, "bcol": bcol}


def kernel(c_input, As, bs, As_inv, _trace=False, _nc_cache={}):
    import ml_dtypes

    c_input = np.asarray(c_input, dtype=np.float32)
    As = np.asarray(As, dtype=np.float32)
    bs = np.asarray(bs, dtype=np.float32)
    As_inv = np.asarray(As_inv, dtype=np.float32)

    np_wdt = ml_dtypes.bfloat16
    if "nc" not in _nc_cache:
        _nc_cache["nc"] = build_program()
    nc = _nc_cache["nc"]

    in_maps = []
    for core in range(NCORES):
        sl = slice(core * NB, (core + 1) * NB)
        in_maps.append(
            _prep_core_inputs(
                c_input[sl], As[sl], bs[sl], As_inv[sl], NB, np_wdt=np_wdt
            )
        )

    res = run_bass_kernel_spmd(nc, in_maps, core_ids=list(range(NCORES)), trace=_trace)

    out = np.empty((B, D), dtype=np.float32)
    for core in range(NCORES):
        oc = res.results[core]["out"]  # [128, NB*5]
        out[core * NB : (core + 1) * NB] = (
            oc.reshape(128, NB, 5).transpose(1, 2, 0).reshape(NB, D)
        )
    if _trace:
        kernel.last_exec_time_ns = res.exec_time_ns
    return out
